# revision 20
# baseline (speedup 1.0000x reference)
"""DynamicConv1D Trainium2 kernel.

Reference computation (per batch b, position s):
    kern[s, h, i] = sum_c x[s, c] * W_pred[c, h*7+i] + b_pred[h*7+i]
    out[s, h, d]  = sum_i kern[s, h, i] * x_pad[s + i, h, d]     (pad = 3)

Sharding: 8 cores = (batch 4) x (sequence halves 2). Each core receives the
transposed shard xT [C=1024, 1030] (1024 positions + 3-halo each side,
zero-padded at sequence ends) and produces outT [1024, 1024] = out.T.

Per-core pipeline:
  1. kern matmul on PE: lhsT = W chunk [128c, 112], rhs = xT chunk
     [128c, 512pos] -> PSUM [112, 512], accumulated over 8 c-chunks.
     ACT copies PSUM -> SBUF with per-partition bias b_pred.
  2. broadcast kern -> kb on PE: selection matmul with constant 0/1 matrix
     sel[:, t, i, :] [112, 128] (fp32r: full-rate single-pass) so
     kb[c, s] = kern[(2t + c//64)*7 + i, s] lands in PSUM [128, 512].
  3. 7 products kb_i * xT[c, s+i] and a 6-add tree on DVE/ACT (nc.any).
"""

import os
import sys

for _p in ("/opt/trn_rl_repo",):
    if _p not in sys.path and os.path.isdir(_p):
        sys.path.insert(0, _p)

import numpy as np

import concourse.bass as bass
import concourse.mybir as mybir
from concourse import tile
from concourse.bass_utils import run_bass_kernel_spmd

B = 4
S = 2048
C = 1024
H = 16
K = 7
HD = 64
PAD = K // 2
KH = K * H  # 112

N_CORES = 8
SHARD = S // 2          # positions per core = 1024
SH = SHARD + 2 * PAD    # 1030 source positions incl halo
NT = C // 128           # 8 channel tiles
SW = 512                # free-dim tile (one PSUM bank of fp32)
NS = SHARD // SW        # 2 position tiles

F32 = mybir.dt.float32
F32R = mybir.dt.float32r

# fp32 main matmul is exact; fp32r is 4x faster (full rate at N>=256) but
# single-pass.  Flip after measuring accuracy on hardware.
MAIN_MM_DTYPE = "f32"


def _build_sel() -> np.ndarray:
    """sel[p, t, i, c] = 1 iff p == (2t + c//64)*7 + i  (p in [0,112))."""
    sel = np.zeros((KH, NT, K, 128), dtype=np.float32)
    for t in range(NT):
        for i in range(K):
            for hh in range(2):  # two heads per 128-channel tile
                p = (2 * t + hh) * K + i
                sel[p, t, i, 64 * hh:64 * (hh + 1)] = 1.0
    return sel


def build_program() -> bass.Bass:
    nc = bass.Bass(trn_type="TRN2")

    xt_d = nc.dram_tensor("xt", [C, SH], F32, kind="ExternalInput")
    wb_d = nc.dram_tensor("wb", [128, NT * KH + 1], F32, kind="ExternalInput")
    sel_d = nc.dram_tensor("sel", [KH, NT * K * 128], F32R, kind="ExternalInput")
    out_d = nc.dram_tensor("out", [C, SHARD], F32, kind="ExternalOutput")

    with tile.TileContext(nc) as tc:
        with (
            tc.tile_pool(name="xt", bufs=1) as xt_pool,
            tc.tile_pool(name="wgt", bufs=1) as w_pool,
            tc.tile_pool(name="sel", bufs=1) as sel_pool,
            tc.tile_pool(name="kern", bufs=1) as kern_pool,
            tc.tile_pool(name="kps", bufs=2, space="PSUM") as kps_pool,
            tc.tile_pool(name="kbps", bufs=2, space="PSUM") as kb_pool,
            tc.tile_pool(name="prod", bufs=10) as prod_pool,
            tc.tile_pool(name="accs", bufs=4) as acc_pool,
            tc.tile_pool(name="prime", bufs=1) as prime_pool,
            tc.tile_pool(name="primeps", bufs=1, space="PSUM") as prime_ps,
        ):
            # ---- loads ----
            # One DMA per tensor: each matmul may carry at most ONE semaphore
            # wait (walrus puts matmul waits on the 1-slot LDWEIGHTS struct),
            # so inputs must each complete on a single DMA sem lane.
            xt_sb = xt_pool.tile([128, NT, SH], F32)
            nc.sync.dma_start(
                xt_sb[:, :, :], xt_d.ap().rearrange("(t p) s -> p t s", p=128)
            )
            wb_sb = w_pool.tile([128, NT * KH + 1], F32)
            nc.scalar.dma_start(wb_sb[:, :], wb_d[:, :])
            sel_sb = sel_pool.tile([KH, NT, K, 128], F32R)
            nc.scalar.dma_start(
                sel_sb[:, :, :, :].rearrange("p t i c -> p (t i c)"), sel_d[:, :]
            )

            # ---- semaphore-lane primers ----
            # Tiny ops that make each engine observe each DMA lane once, so
            # every subsequent real instruction needs at most one wait
            # (walrus allows only one sync wait per matmul/LDWEIGHTS).
            tiny = prime_pool.tile([1, 8], F32)
            tiny_ps = prime_ps.tile([2, 16], F32)
            nc.tensor.matmul(tiny_ps[0:1, 0:1], wb_sb[0:1, 0:1],
                             wb_sb[0:1, 0:1], start=True, stop=True)
            nc.tensor.matmul(tiny_ps[0:1, 1:2], xt_sb[0:1, 0, 0:1],
                             xt_sb[0:1, 0, 0:1], start=True, stop=True)
            nc.tensor.matmul(tiny_ps[0:2, 2:4], sel_sb[0:2, 0, 0, 0:2],
                             sel_sb[0:2, 0, 0, 0:2], start=True, stop=True)
            nc.scalar.copy(tiny[:, 0:1], wb_sb[0:1, NT * KH:NT * KH + 1])
            nc.vector.tensor_copy(tiny[:, 1:2], xt_sb[0:1, 0, 0:1])

            # ---- kern = x @ W + b ----
            kern_sb = kern_pool.tile([KH, SHARD], F32R)
            for sj in range(NS):
                kps = kps_pool.tile([KH, SW], F32)
                for m in range(NT):
                    if MAIN_MM_DTYPE == "f32r":
                        lhsT = wb_sb[:, KH * m:KH * (m + 1)].bitcast(F32R)
                        rhs = xt_sb[:, m, PAD + SW * sj:PAD + SW * sj + SW].bitcast(F32R)
                    else:
                        lhsT = wb_sb[:, KH * m:KH * (m + 1)]
                        rhs = xt_sb[:, m, PAD + SW * sj:PAD + SW * sj + SW]
                    nc.tensor.matmul(
                        kps[:, :], lhsT, rhs, start=(m == 0), stop=(m == NT - 1)
                    )
                # PSUM -> SBUF, adding the per-partition bias
                nc.scalar.activation(
                    kern_sb[:, SW * sj:SW * (sj + 1)], kps[:, :],
                    mybir.ActivationFunctionType.Identity, bias=wb_sb[0:KH, NT * KH:NT * KH + 1],
                )

            # Primer so PE observes both kern_sb writes (one ACT semaphore
            # covers both halves) before the selection matmuls.
            nc.tensor.matmul(tiny_ps[0:2, 8:16], kern_sb[0:2, SW - 2:SW],
                             kern_sb[0:2, SW - 4:SW + 4], start=True, stop=True)

            # ---- dynamic conv ----
            # Both 512-position halves are processed together: the two kb
            # matmuls for tap i land in one 2-bank PSUM tile [128, 2*SW], and
            # products/adds run at 1024-free to amortize the per-op overhead.
            FW = NS * SW  # 1024
            for g in range(NT // 4):  # 4 channel tiles share one out-DMA
                stage = acc_pool.tile([128, 4, FW], F32, tag="stage")
                for u in range(4):
                    t = 4 * g + u
                    prods = []
                    for i in range(K):
                        kb = kb_pool.tile([128, FW], F32)
                        for sj in range(NS):
                            nc.tensor.matmul(
                                kb[:, SW * sj:SW * (sj + 1)],
                                sel_sb[:, t, i, :],
                                kern_sb[:, SW * sj:SW * (sj + 1)],
                                start=True, stop=True,
                            )
                        p = prod_pool.tile([128, FW], F32, tag="prod")
                        # x slice for tap i covers both halves contiguously
                        nc.any.tensor_mul(p[:, :], kb[:, :], xt_sb[:, t, i:i + FW])
                        prods.append(p)
                    # 6-add tree at 1024-free
                    a01 = acc_pool.tile([128, FW], F32, tag="acc")
                    nc.any.tensor_add(a01[:, :], prods[0][:, :], prods[1][:, :])
                    a23 = acc_pool.tile([128, FW], F32, tag="acc")
                    nc.any.tensor_add(a23[:, :], prods[2][:, :], prods[3][:, :])
                    a45 = acc_pool.tile([128, FW], F32, tag="acc")
                    nc.any.tensor_add(a45[:, :], prods[4][:, :], prods[5][:, :])
                    a03 = acc_pool.tile([128, FW], F32, tag="acc")
                    nc.any.tensor_add(a03[:, :], a01[:, :], a23[:, :])
                    a46 = acc_pool.tile([128, FW], F32, tag="acc")
                    nc.any.tensor_add(a46[:, :], a45[:, :], prods[6][:, :])
                    nc.any.tensor_add(stage[:, u, :], a03[:, :], a46[:, :])
                nc.sync.dma_start(
                    out_d.ap().rearrange("(g u p) s -> g p u s", u=4, p=128)[g],
                    stage[:, :, :],
                )

    _strip_same_engine_waits(nc)
    return nc


# Engines complete their own instructions in program order (PE matmuls are
# pc-monotone in start AND end), so a wait on the engine's own completion
# semaphore is always satisfied by program order.  Tile still emits them for
# PSUM-slot WAW tracking; walrus then rejects matmuls with >1 wait (the
# LDWEIGHTS struct has a single sync-wait slot).  Strip them.
def _strip_same_engine_waits(nc: bass.Bass) -> None:
    # (1) PE matmuls complete in strict pc order (silicon), so a wait on PE's
    # own completion semaphore is redundant -> strip, keeping each matmul at
    # <=1 wait (walrus LDWEIGHTS has a single sync-wait slot).
    # (2) The exit Drain waits on every semaphore ever used, exceeding the
    # struct's wait capacity.  Input-DMA lane waits are covered transitively:
    # each compute engine waited on those lanes before its last instruction,
    # and the Drain still waits on every engine's final count.  Keep only
    # engine sems and the out-DMA lanes (nothing else observes those).
    out_lanes = set()
    for blk in nc.m.functions[0].blocks:
        for inst in blk.instructions:
            if inst.opcode != "DMACopy":
                continue
            dst = inst.outs[0]
            if getattr(dst, "memref", "").startswith("out"):
                for u in (inst.sync_info.on_update if inst.sync_info else []):
                    out_lanes.add(u.ant_name)
    noop_n = [0]
    for blk in nc.m.functions[0].blocks:
        for inst in blk.instructions:
            si = inst.sync_info
            if si is None or not si.on_wait:
                continue
            if str(inst.engine) == "EngineType.PE":
                kept = [w for w in si.on_wait if not w.ant_name.startswith("PE_")]
            elif inst.opcode == "Drain":
                kept = [
                    w for w in si.on_wait
                    if not w.ant_name.startswith("DMAHW") or w.ant_name in out_lanes
                ]
            else:
                continue
            if len(kept) != len(si.on_wait):
                inst.sync_info = mybir.SyncInfo(
                    on_wait=kept, on_update=list(si.on_update)
                )
    # Any instruction still carrying >1 wait: keep the first wait and move the
    # extras onto single-wait NoOps inserted just before it (same engine) --
    # the walrus instruction structs have a single sync-wait slot.
    for blk in nc.m.functions[0].blocks:
        il = blk.instructions
        idx = 0
        while idx < len(il):
            inst = il[idx]
            si = inst.sync_info
            if si is not None and len(si.on_wait) > 1:
                waits = list(si.on_wait)
                for w in waits[:-1]:
                    noop_n[0] += 1
                    nop = mybir.InstNoOp(
                        name=f"I-waitsplit-{noop_n[0]}",
                        engine=inst.engine,
                        ins=[], outs=[],
                        sync_info=mybir.SyncInfo(on_wait=[w], on_update=[]),
                    )
                    nc.register_instruction(nop, overwrite=True)
                    il.insert(idx, nop)
                    idx += 1
                inst.sync_info = mybir.SyncInfo(
                    on_wait=[waits[-1]], on_update=list(si.on_update)
                )
            idx += 1


_PROGRAM = None


def _get_program() -> bass.Bass:
    global _PROGRAM
    if _PROGRAM is None:
        _PROGRAM = build_program()
    return _PROGRAM


def make_in_maps(x: np.ndarray, W_pred: np.ndarray, b_pred: np.ndarray):
    sel = np.ascontiguousarray(_build_sel().reshape(KH, NT * K * 128))
    # wb blob: [p, t*KH + k] = W_pred[t*128 + p, k]; last column = b_pred
    wb = np.zeros((128, NT * KH + 1), dtype=np.float32)
    wb[:, :NT * KH] = (
        np.asarray(W_pred, dtype=np.float32)
        .reshape(NT, 128, KH).transpose(1, 0, 2).reshape(128, NT * KH)
    )
    wb[:KH, NT * KH] = np.asarray(b_pred, dtype=np.float32)
    in_maps = []
    for core in range(N_CORES):
        b_idx, half = divmod(core, 2)
        s0 = half * SHARD
        xp = np.zeros((SH, C), dtype=np.float32)
        lo = max(0, s0 - PAD)
        hi = min(S, s0 + SHARD + PAD)
        xp[lo - (s0 - PAD):hi - (s0 - PAD)] = x[b_idx, lo:hi]
        xt = np.ascontiguousarray(xp.T)
        in_maps.append({"xt": xt, "wb": wb, "sel": sel})
    return in_maps


def assemble(results) -> np.ndarray:
    out = np.empty((B, S, C), dtype=np.float32)
    for core in range(N_CORES):
        b_idx, half = divmod(core, 2)
        out[b_idx, half * SHARD:(half + 1) * SHARD] = results[core]["out"].T
    return out


def kernel(x: np.ndarray, W_pred: np.ndarray, b_pred: np.ndarray) -> np.ndarray:
    nc = _get_program()
    in_maps = make_in_maps(np.asarray(x), np.asarray(W_pred), np.asarray(b_pred))
    res = run_bass_kernel_spmd(nc, in_maps, list(range(N_CORES)))
    return assemble(res.results)
